# revision 43
# baseline (speedup 1.0000x reference)
"""Trainium2 Bass kernel for nn_AttentionBlock (linear attention + BatchNorm).

Math (per batch, c=256 channels, n=1024 pixels, 8 heads x 64 dims):
  qkv = w_qkv @ x                      [1536, n]
  q   = softmax(q, axis=d) * d^-0.5    (per head, over the 64 head-dims)
  k   = softmax(k, axis=n)             (per head-dim, over pixels)
  ctx = k @ (v/n)^T                    [d, e] per head
  out = ctx^T @ q                      [e, n] per head
  y   = BatchNorm(w_out @ out + b_out) (batch stats over (b, n) per channel)

Sharding: data-parallel over batch across 8 cores (4 batches each).  BN
batch stats (2 f32 per channel) are combined with one small ncfw AllReduce;
two dummy AllReduces issued at program start hide the cold-start cost (the
3rd collective runs ~14us warm) and force a coordinated 8-core launch.
b_out is skipped: BatchNorm's mean subtraction cancels any per-channel
constant exactly.  y is stored fp16 (host upconverts), halving output DMA.

Per-batch device schedule (PE issue order, dense):
  KV(t=0..7)  w/ ctx pass A (head pairs 0,1) interleaved at lag 1
  Q(tq=0..3)  [128,1024] psum tiles, 1024-wide exp
  ctx pass B  (head pairs 2,3; re-reads resident expk/vx SBUF tiles)
  Zq+OUT(tq)  Zq via block-mask matmul; OUT via block-diag packed lhsT
              (one [128,128] matmul computes both heads of the pair)
  FINAL       k4-major accumulation so it pipelines with the out/Zq divides
Engines: ACT = exps, v-casts (even t), psum->sbuf final copies; DVE = Zq
reciprocal + out multiply, ctx normalize, bn_stats, v-casts (odd t);
Pool = memsets + tail normalize share (Pool cannot touch PSUM on TRN2).
Hardware lessons baked in: DVE has no divide and reads at most one PSUM
operand per op; partition-offset reciprocal misbehaves (the ctx ones-column
gives Zk for all 128 partitions, so one full recip suffices); raw SBUF-to-
SBUF remote DMA packetizes per partition (16B packets) and is far slower
than the warmed ncfw collective for this shape.
"""

import os
import sys

import numpy as np

for _p in ("/opt/trn_rl_repo", "/root/.axon_site/_ro/trn_rl_repo"):
    if os.path.isdir(_p) and _p not in sys.path:
        sys.path.insert(0, _p)

import concourse.bacc as bacc
import concourse.tile as tile
from concourse import mybir
from concourse.bass_utils import run_bass_kernel_spmd

F32 = mybir.dt.float32
FP16 = mybir.dt.float16
AF = mybir.ActivationFunctionType
ALU = mybir.AluOpType

N_CORES = 8
# B is overridable for cheap simulator runs (BASS_ATTN_B=1 -> 8 batches total).
B = int(os.environ.get("BASS_ATTN_B", "4"))  # batches per core
C = 256          # channels
NPIX = 1024      # pixels (32*32)
H = 8            # heads
D = 64           # head dim
HID = H * D      # 512
NT = NPIX // 128  # 8 n-tiles
CT = C // 128     # 2 c-tiles
QT = HID // 128   # 4 q-tiles (also the number of head pairs)
SCALE = D ** -0.5
BN_EPS = 1e-5
# Zq-broadcast matmul uses this instead of 1.0 so outp / zqb directly
# yields out * SCALE / (n * Zq), folding the softmax scale and the v/n factor.
MASKVAL = NPIX / SCALE


def _emit(tc, x, wqkv, wout, gammab, betab, y):
    nc = tc.nc
    from contextlib import ExitStack
    ctx_stack = ExitStack()
    with ctx_stack:
        const = ctx_stack.enter_context(tc.tile_pool(name="const", bufs=1))
        xin = ctx_stack.enter_context(tc.tile_pool(name="xin", bufs=4))
        kpool = ctx_stack.enter_context(tc.tile_pool(name="kpool", bufs=10))
        vpool = ctx_stack.enter_context(tc.tile_pool(name="vpool", bufs=10))
        qpool = ctx_stack.enter_context(tc.tile_pool(name="qpool", bufs=6))
        bdpool = ctx_stack.enter_context(tc.tile_pool(name="bdpool", bufs=6))
        ospool = ctx_stack.enter_context(tc.tile_pool(name="ospool", bufs=6))
        zspool = ctx_stack.enter_context(tc.tile_pool(name="zspool", bufs=4))
        small = ctx_stack.enter_context(tc.tile_pool(name="small", bufs=12))
        fpool = ctx_stack.enter_context(tc.tile_pool(name="fpool", bufs=2 * B))
        ypool = ctx_stack.enter_context(tc.tile_pool(name="ypool", bufs=2 * B))
        stats_p = ctx_stack.enter_context(tc.tile_pool(name="statsp", bufs=1))
        pbig = ctx_stack.enter_context(
            tc.tile_pool(name="pbig", bufs=3, space="PSUM"))
        pctx = ctx_stack.enter_context(
            tc.tile_pool(name="pctx", bufs=2, space="PSUM"))
        dpool = ctx_stack.enter_context(
            tc.tile_pool(name="dram", bufs=1, space="DRAM"))

        # ---- constants ----
        wqkv_sb = []
        for kc in range(CT):
            w = const.tile([128, 3 * HID], FP16, name=f"wqkv{kc}")
            # kv columns first so the first batch's kv matmuls start early
            nc.sync.dma_start(out=w[:, HID:3 * HID],
                              in_=wqkv[128 * kc:128 * (kc + 1), HID:3 * HID])
            nc.sync.dma_start(out=w[:, 0:HID],
                              in_=wqkv[128 * kc:128 * (kc + 1), 0:HID])
            wqkv_sb.append(w)
        wout_sb = []
        for k4 in range(HID // 128):
            w = const.tile([128, C], FP16, name=f"wout{k4}")
            nc.sync.dma_start(out=w, in_=wout[128 * k4:128 * (k4 + 1), :])
            wout_sb.append(w)
        gamma_sb, beta_sb = [], []
        for m in range(CT):
            g = const.tile([128, 1], F32, name=f"gamma{m}")
            nc.sync.dma_start(out=g, in_=gammab[128 * m:128 * (m + 1), :])
            gamma_sb.append(g)
            bb = const.tile([128, 1], F32, name=f"beta{m}")
            nc.sync.dma_start(out=bb, in_=betab[128 * m:128 * (m + 1), :])
            beta_sb.append(bb)
        bmask = const.tile([128, 128], FP16, name="bmask")
        nc.vector.memset(bmask, 0.0)
        nc.vector.memset(bmask[0:64, 0:64], MASKVAL)
        nc.vector.memset(bmask[64:128, 64:128], MASKVAL)
        eps_sb = const.tile([128, 1], F32, name="eps")
        nc.vector.memset(eps_sb, BN_EPS)
        pk4 = const.tile([128, 2 * CT], F32, name="pk4")

        # Dummy AllReduce: forces a coordinated collective launch across the
        # 8 cores (without it the runtime serializes core executions and the
        # stats-RDMA arrival wait spans other cores' entire runs) and acts as
        # the start rendezvous for the raw RDMA protocol.  Runs concurrently
        # with compute; result unused.
        wrm_i = dpool.tile([128, 1], F32, name="wrm_i")
        wrm_o = dpool.tile([128, 1], F32, name="wrm_o")
        wrm_s = const.tile([128, 1], F32, name="wrm_s")
        nc.vector.memset(wrm_s, 0.0)
        nc.sync.dma_start(out=wrm_i, in_=wrm_s)
        for _ in range(2):
            nc.gpsimd.collective_compute(
                "AllReduce", ALU.add,
                replica_groups=[list(range(N_CORES))],
                ins=[wrm_i.opt()], outs=[wrm_o.opt()])

        stats_sb = [stats_p.tile([128, 2 * B, 6], F32, name=f"stats{m}")
                    for m in range(CT)]
        # one contiguous [128, B*1024] final buffer per channel group so the
        # tail normalize is a single wide op per m
        fs_m = [fpool.tile([128, B * NPIX], F32, name=f"fsm{m}", bufs=1)
                for m in range(CT)]
        yv_m = [ypool.tile([128, B * NPIX], FP16, name=f"yvm{m}", bufs=1)
                for m in range(CT)]

        # ---- per-batch compute ----
        xc_all = [None] * B

        def load_x(b):
            xc = []
            for kc in range(CT):
                xt = xin.tile([128, NPIX], FP16, name="xc")
                nc.scalar.dma_start(
                    out=xt, in_=x[b, 128 * kc:128 * (kc + 1), :])
                xc.append(xt)
            xc_all[b] = xc

        load_x(0)
        pend_kv = {}

        def emit_kv_tile(xc, t):
            """KV-projection matmuls for one n-tile + exp(k) + v cast."""
            kvp = pbig.tile([128, 1024], F32, name="kvp", tag="big")
            for kc in range(CT):
                nc.tensor.matmul(
                    kvp[:, 0:512],
                    lhsT=xc[kc][:, 128 * t:128 * (t + 1)],
                    rhs=wqkv_sb[kc][:, HID:2 * HID],
                    start=(kc == 0), stop=(kc == CT - 1))
            for kc in range(CT):
                nc.tensor.matmul(
                    kvp[:, 512:1024],
                    lhsT=xc[kc][:, 128 * t:128 * (t + 1)],
                    rhs=wqkv_sb[kc][:, 2 * HID:3 * HID],
                    start=(kc == 0), stop=(kc == CT - 1))
            ek = kpool.tile([128, HID], FP16, name="expk")
            nc.scalar.activation(out=ek, in_=kvp[:, 0:512], func=AF.Exp)
            vt = vpool.tile([128, H, D + 1], FP16, name="vx")
            nc.gpsimd.memset(vt[:, :, D:D + 1], 1.0)
            vdst = vt[:, :, 0:D]
            vsrc = kvp[:, 512:1024].rearrange("p (h e) -> p h e", h=H)
            if t % 2 == 0:
                nc.scalar.copy(vdst, vsrc)
            else:
                nc.vector.tensor_copy(vdst, vsrc)
            return ek, vt

        for b in range(B):
            xc = xc_all[b]
            pend = pend_kv.pop(b, None)
            expk = [None] * NT
            vx = [None] * NT
            ctxu = [None] * QT   # [128, 130] psum per head pair

            def ctx_mm(t, pr):
                if ctxu[pr] is None:
                    ctxu[pr] = pctx.tile([128, 2 * (D + 1)], F32,
                                         name="ctxu", tag="ctx")
                nc.tensor.matmul(
                    ctxu[pr],
                    lhsT=expk[t][:, 128 * pr:128 * (pr + 1)],
                    rhs=vx[t][:, 2 * pr:2 * (pr + 1), :],
                    start=(t == 0), stop=(t == NT - 1))

            # ---- KV projection; ctx pass A (head pairs 0,1) at lag 1 ----
            for t in range(NT):
                if pend is not None and t < len(pend):
                    expk[t], vx[t] = pend[t]
                else:
                    expk[t], vx[t] = emit_kv_tile(xc, t)
                if t >= 1:
                    ctx_mm(t - 1, 0)
                    ctx_mm(t - 1, 1)
            ctx_mm(NT - 1, 0)
            ctx_mm(NT - 1, 1)

            # prefetch next batch's x while this batch computes
            if b + 1 < B:
                load_x(b + 1)

            # ---- Q projection (1024-wide psum tiles + exp) ----
            expq = [None] * QT
            for tq in range(QT):
                qp = pbig.tile([128, 1024], F32, name="qp", tag="big")
                for nch in range(2):
                    for kc in range(CT):
                        nc.tensor.matmul(
                            qp[:, 512 * nch:512 * (nch + 1)],
                            lhsT=wqkv_sb[kc][:, 128 * tq:128 * (tq + 1)],
                            rhs=xc[kc][:, 512 * nch:512 * (nch + 1)],
                            start=(kc == 0), stop=(kc == CT - 1))
                eq = qpool.tile([128, 1024], FP16, name="expq")
                nc.scalar.activation(out=eq, in_=qp, func=AF.Exp)
                expq[tq] = eq

            # ---- ctx normalize + block-diag lhsT build (pass A pairs) ----
            bd = [None] * QT

            def extract(pr, alt):
                bdt = bdpool.tile([128, 128], FP16, name="bd")
                nc.gpsimd.memset(bdt, 0.0)
                cu = ctxu[pr]
                # the ones column multiplies every lhsT row, so ctxu[:, D]
                # already holds Zk for BOTH heads' partitions -- one
                # full-partition reciprocal (partition-offset DVE ops
                # misbehave on hardware)
                rz = small.tile([128, 1], F32, name="rz")
                nc.vector.reciprocal_approx_fast(
                    out=rz, in_=cu[:, D:D + 1])
                nc.vector.tensor_scalar_mul(
                    bdt[0:64, 0:64], in0=cu[0:64, 0:D], scalar1=rz[0:64, :])
                nc.vector.tensor_scalar_mul(
                    bdt[64:128, 64:128], in0=cu[64:128, D + 1:2 * D + 1],
                    scalar1=rz[64:128, :])
                bd[pr] = bdt

            extract(0, False)
            extract(1, True)

            # ---- ctx pass B (head pairs 2,3) ----
            ctxu[0] = ctxu[1] = None  # pass B allocates fresh ring slots
            for t in range(NT):
                ctx_mm(t, 2)
                ctx_mm(t, 3)
            extract(2, False)
            extract(3, True)

            # ---- Zq broadcast + OUT (block-diag packed) + divide ----
            out_sb = []
            for tq in range(QT):
                zqb = pbig.tile([128, 1024], F32, name="zqb", tag="big")
                outp = pbig.tile([128, 1024], F32, name="outp", tag="big")
                for nch in range(2):
                    nc.tensor.matmul(
                        zqb[:, 512 * nch:512 * (nch + 1)],
                        lhsT=bmask,
                        rhs=expq[tq][:, 512 * nch:512 * (nch + 1)],
                        start=True, stop=True)
                for nch in range(2):
                    nc.tensor.matmul(
                        outp[:, 512 * nch:512 * (nch + 1)],
                        lhsT=bd[tq],
                        rhs=expq[tq][:, 512 * nch:512 * (nch + 1)],
                        start=True, stop=True)
                # No divide in the DVE ISA, and only one operand may come
                # from PSUM: reciprocal(zqb) -> SBUF f32, then outp * recip.
                rzb = zspool.tile([128, 1024], F32, name="rzb")
                nc.vector.reciprocal_approx_fast(out=rzb, in_=zqb)
                os_ = ospool.tile([128, 1024], FP16, name="outsb")
                nc.vector.tensor_mul(os_, outp, rzb)
                out_sb.append(os_)

            # fill the FINAL-phase recip/mul stall with the next batch's
            # first two KV tiles (software pipelining across batches)
            if b + 1 < B:
                pend_kv[b + 1] = [emit_kv_tile(xc_all[b + 1], t)
                                  for t in range(2)]

            # warm the sqrt activation table once all exps are done: the BN
            # tail's Sqrt must not pay the ~1.3us act-table load
            if b == B - 1:
                warm_sq = small.tile([1, 1], F32, name="warmsq")
                nc.scalar.activation(out=warm_sq, in_=eps_sb[0:1, :],
                                     func=AF.Sqrt)

            # ---- final projection (k4-major), bn stats, psum->sbuf copy ----
            fh = [pbig.tile([128, 1024], F32, name="fh", tag="big")
                  for _ in range(CT)]
            for k4 in range(HID // 128):
                for m in range(CT):
                    for nch in range(2):
                        nc.tensor.matmul(
                            fh[m][:, 512 * nch:512 * (nch + 1)],
                            lhsT=wout_sb[k4][:, 128 * m:128 * (m + 1)],
                            rhs=out_sb[k4][:, 512 * nch:512 * (nch + 1)],
                            start=(k4 == 0), stop=(k4 == HID // 128 - 1))
            for m in range(CT):
                for nch in range(2):
                    nc.vector.bn_stats(
                        out=stats_sb[m][:, 2 * b + nch, :],
                        in_=fh[m][:, 512 * nch:512 * (nch + 1)])
                nc.scalar.copy(
                    fs_m[m][:, NPIX * b:NPIX * (b + 1)], fh[m])

        # ---- batch-norm stats: local aggregate + RDMA all-exchange ----
        for m in range(CT):
            mv = small.tile([128, 2], F32, name="mv")
            nc.vector.bn_aggr(out=mv, in_=stats_sb[m])
            pk = pk4[:, 2 * m:2 * (m + 1)]
            nc.vector.tensor_mul(pk[:, 1:2], mv[:, 0:1], mv[:, 0:1])
            nc.vector.tensor_add(pk[:, 1:2], pk[:, 1:2], mv[:, 1:2])
            nc.vector.tensor_copy(pk[:, 0:1], mv[:, 0:1])
            nc.vector.tensor_scalar_mul(pk, in0=pk, scalar1=1.0 / N_CORES)

        ccin = dpool.tile([128, 2 * CT], F32, name="ccin")
        ccout = dpool.tile([128, 2 * CT], F32, name="ccout")
        nc.sync.dma_start(out=ccin, in_=pk4)
        nc.gpsimd.collective_compute(
            "AllReduce", ALU.add,
            replica_groups=[list(range(N_CORES))],
            ins=[ccin.opt()], outs=[ccout.opt()])
        gst = small.tile([128, 2 * CT], F32, name="gst")
        nc.sync.dma_start(out=gst, in_=ccout)

        # ---- normalize + store (in-tile; Tile handles all ordering) ----
        # affine math vectorized over both channel groups ([128, 2] ops,
        # strided views of gst), then ONE wide normalize per m (DVE || ACT)
        gmean2 = gst.rearrange("p (m two) -> p m two", two=2)[:, :, 0]
        gex22 = gst.rearrange("p (m two) -> p m two", two=2)[:, :, 1]
        var2 = small.tile([128, CT], F32, name="var2")
        nc.vector.tensor_mul(var2, gmean2, gmean2)
        nc.vector.tensor_sub(var2, gex22, var2)
        std2 = small.tile([128, CT], F32, name="std2")
        nc.scalar.activation(out=std2, in_=var2, func=AF.Sqrt, bias=eps_sb)
        rstd2 = small.tile([128, CT], F32, name="rstd2")
        nc.vector.reciprocal_approx_fast(out=rstd2, in_=std2)
        gb2 = small.tile([128, 2 * CT], F32, name="gb2")
        for m in range(CT):
            nc.vector.tensor_copy(gb2[:, m:m + 1], gamma_sb[m])
            nc.vector.tensor_copy(gb2[:, CT + m:CT + m + 1], beta_sb[m])
        rsg2 = small.tile([128, CT], F32, name="rsg2")
        nc.vector.tensor_mul(rsg2, rstd2, gb2[:, 0:CT])
        sh2 = small.tile([128, CT], F32, name="sh2")
        nc.vector.tensor_mul(sh2, gmean2, rsg2)
        nc.vector.tensor_sub(sh2, gb2[:, CT:2 * CT], sh2)
        for m in range(CT):
            if m == 0:
                nc.vector.tensor_scalar(
                    out=yv_m[m], in0=fs_m[m],
                    scalar1=rsg2[:, m:m + 1], scalar2=sh2[:, m:m + 1],
                    op0=ALU.mult, op1=ALU.add)
            else:
                nc.scalar.activation(
                    out=yv_m[m], in_=fs_m[m], func=AF.Identity,
                    bias=sh2[:, m:m + 1], scale=rsg2[:, m:m + 1])
            for b in range(B):
                eng = nc.sync if b % 2 == 0 else nc.scalar
                eng.dma_start(out=y[b, 128 * m:128 * (m + 1), :],
                              in_=yv_m[m][:, NPIX * b:NPIX * (b + 1)])


_CACHE = {}


def _build():
    if "nc" in _CACHE:
        return _CACHE["nc"]
    nc = bacc.Bacc("TRN2", target_bir_lowering=False, debug=False,
                   enable_asserts=True, num_devices=N_CORES)
    x = nc.dram_tensor("x", [B, C, NPIX], FP16, kind="ExternalInput").ap()
    wqkv = nc.dram_tensor("wqkvT", [C, 3 * HID], FP16,
                          kind="ExternalInput").ap()
    wout = nc.dram_tensor("woutT", [HID, C], FP16, kind="ExternalInput").ap()
    gammab = nc.dram_tensor("gammab", [C, 1], F32, kind="ExternalInput").ap()
    betab = nc.dram_tensor("betab", [C, 1], F32, kind="ExternalInput").ap()
    y = nc.dram_tensor("y", [B, C, NPIX], FP16, kind="ExternalOutput").ap()

    with tile.TileContext(nc) as tc:
        _emit(tc, x, wqkv, wout, gammab, betab, y)
    nc.compile()
    _CACHE["nc"] = nc
    return nc


def kernel(x, w_qkv, w_out, b_out, gamma, beta, _trace=False):
    x = np.asarray(x, dtype=np.float32)
    wqkvT = np.ascontiguousarray(np.asarray(w_qkv, np.float16).T)   # [256, 1536]
    woutT = np.ascontiguousarray(np.asarray(w_out, np.float16).T)   # [512, 256]
    gammab = np.ascontiguousarray(np.asarray(gamma, np.float32).reshape(C, 1))
    betab = np.ascontiguousarray(np.asarray(beta, np.float32).reshape(C, 1))
    # b_out is intentionally unused: BatchNorm's mean subtraction cancels any
    # per-channel constant added before it, exactly.

    btot, c, hh, ww = x.shape
    assert (btot, c, hh * ww) == (B * N_CORES, C, NPIX)
    xf = x.reshape(btot, C, NPIX)

    nc = _build()
    in_maps = []
    for core in range(N_CORES):
        in_maps.append({
            "x": np.ascontiguousarray(xf[B * core:B * (core + 1)]).astype(np.float16),
            "wqkvT": wqkvT,
            "woutT": woutT,
            "gammab": gammab,
            "betab": betab,
        })
    res = run_bass_kernel_spmd(nc, in_maps, core_ids=list(range(N_CORES)),
                               trace=_trace)
    y = np.concatenate([res.results[core]["y"] for core in range(N_CORES)],
                       axis=0)
    out = y.reshape(btot, C, hh, ww).astype(np.float32)
    if _trace:
        kernel.last_result = res
    return out


# revision 44
# speedup vs baseline: 1.0276x; 1.0276x over previous
"""Trainium2 Bass kernel for nn_AttentionBlock (linear attention + BatchNorm).

Math (per batch, c=256 channels, n=1024 pixels, 8 heads x 64 dims):
  qkv = w_qkv @ x                      [1536, n]
  q   = softmax(q, axis=d) * d^-0.5    (per head, over the 64 head-dims)
  k   = softmax(k, axis=n)             (per head-dim, over pixels)
  ctx = k @ (v/n)^T                    [d, e] per head
  out = ctx^T @ q                      [e, n] per head
  y   = BatchNorm(w_out @ out + b_out) (batch stats over (b, n) per channel)

Sharding: data-parallel over batch across 8 cores (4 batches each).  BN
batch stats (2 f32 per channel) are combined with one small ncfw AllReduce;
two dummy AllReduces issued at program start hide the cold-start cost (the
3rd collective runs ~14us warm) and force a coordinated 8-core launch.
b_out is skipped: BatchNorm's mean subtraction cancels any per-channel
constant exactly.  y is stored fp16 (host upconverts), halving output DMA.

Per-batch device schedule (PE issue order, dense):
  KV(t=0..7)  w/ ctx pass A (head pairs 0,1) interleaved at lag 1
  Q(tq=0..3)  [128,1024] psum tiles, 1024-wide exp
  ctx pass B  (head pairs 2,3; re-reads resident expk/vx SBUF tiles)
  Zq+OUT(tq)  Zq via block-mask matmul; OUT via block-diag packed lhsT
              (one [128,128] matmul computes both heads of the pair)
  FINAL       k4-major accumulation so it pipelines with the out/Zq divides
Engines: ACT = exps, v-casts (even t), psum->sbuf final copies; DVE = Zq
reciprocal + out multiply, ctx normalize, bn_stats, v-casts (odd t);
Pool = memsets + tail normalize share (Pool cannot touch PSUM on TRN2).
Hardware lessons baked in: DVE has no divide and reads at most one PSUM
operand per op; partition-offset reciprocal misbehaves (the ctx ones-column
gives Zk for all 128 partitions, so one full recip suffices); raw SBUF-to-
SBUF remote DMA packetizes per partition (16B packets) and is far slower
than the warmed ncfw collective for this shape.
"""

import os
import sys

import numpy as np

for _p in ("/opt/trn_rl_repo", "/root/.axon_site/_ro/trn_rl_repo"):
    if os.path.isdir(_p) and _p not in sys.path:
        sys.path.insert(0, _p)

import concourse.bacc as bacc
import concourse.tile as tile
from concourse import mybir
from concourse.bass_utils import run_bass_kernel_spmd

F32 = mybir.dt.float32
FP16 = mybir.dt.float16
AF = mybir.ActivationFunctionType
ALU = mybir.AluOpType

N_CORES = 8
# B is overridable for cheap simulator runs (BASS_ATTN_B=1 -> 8 batches total).
B = int(os.environ.get("BASS_ATTN_B", "4"))  # batches per core
C = 256          # channels
NPIX = 1024      # pixels (32*32)
H = 8            # heads
D = 64           # head dim
HID = H * D      # 512
NT = NPIX // 128  # 8 n-tiles
CT = C // 128     # 2 c-tiles
QT = HID // 128   # 4 q-tiles (also the number of head pairs)
SCALE = D ** -0.5
BN_EPS = 1e-5
# Zq-broadcast matmul uses this instead of 1.0 so outp / zqb directly
# yields out * SCALE / (n * Zq), folding the softmax scale and the v/n factor.
MASKVAL = NPIX / SCALE


def _emit(tc, x, wqkv, wout, gammab, betab, y):
    nc = tc.nc
    from contextlib import ExitStack
    ctx_stack = ExitStack()
    with ctx_stack:
        const = ctx_stack.enter_context(tc.tile_pool(name="const", bufs=1))
        xin = ctx_stack.enter_context(tc.tile_pool(name="xin", bufs=4))
        kpool = ctx_stack.enter_context(tc.tile_pool(name="kpool", bufs=10))
        vpool = ctx_stack.enter_context(tc.tile_pool(name="vpool", bufs=10))
        qpool = ctx_stack.enter_context(tc.tile_pool(name="qpool", bufs=6))
        bdpool = ctx_stack.enter_context(tc.tile_pool(name="bdpool", bufs=6))
        ospool = ctx_stack.enter_context(tc.tile_pool(name="ospool", bufs=6))
        zspool = ctx_stack.enter_context(tc.tile_pool(name="zspool", bufs=4))
        small = ctx_stack.enter_context(tc.tile_pool(name="small", bufs=12))
        fpool = ctx_stack.enter_context(tc.tile_pool(name="fpool", bufs=2 * B))
        ypool = ctx_stack.enter_context(tc.tile_pool(name="ypool", bufs=2 * B))
        stats_p = ctx_stack.enter_context(tc.tile_pool(name="statsp", bufs=1))
        pbig = ctx_stack.enter_context(
            tc.tile_pool(name="pbig", bufs=3, space="PSUM"))
        pctx = ctx_stack.enter_context(
            tc.tile_pool(name="pctx", bufs=2, space="PSUM"))
        dpool = ctx_stack.enter_context(
            tc.tile_pool(name="dram", bufs=1, space="DRAM"))

        # ---- constants ----
        wqkv_sb = []
        for kc in range(CT):
            w = const.tile([128, 3 * HID], FP16, name=f"wqkv{kc}")
            # kv columns first so the first batch's kv matmuls start early
            nc.sync.dma_start(out=w[:, HID:3 * HID],
                              in_=wqkv[128 * kc:128 * (kc + 1), HID:3 * HID])
            nc.sync.dma_start(out=w[:, 0:HID],
                              in_=wqkv[128 * kc:128 * (kc + 1), 0:HID])
            wqkv_sb.append(w)
        wout_sb = []
        for k4 in range(HID // 128):
            w = const.tile([128, C], FP16, name=f"wout{k4}")
            nc.sync.dma_start(out=w, in_=wout[128 * k4:128 * (k4 + 1), :])
            wout_sb.append(w)
        gamma_sb, beta_sb = [], []
        for m in range(CT):
            g = const.tile([128, 1], F32, name=f"gamma{m}")
            nc.sync.dma_start(out=g, in_=gammab[128 * m:128 * (m + 1), :])
            gamma_sb.append(g)
            bb = const.tile([128, 1], F32, name=f"beta{m}")
            nc.sync.dma_start(out=bb, in_=betab[128 * m:128 * (m + 1), :])
            beta_sb.append(bb)
        bmask = const.tile([128, 128], FP16, name="bmask")
        nc.vector.memset(bmask, 0.0)
        nc.vector.memset(bmask[0:64, 0:64], MASKVAL)
        nc.vector.memset(bmask[64:128, 64:128], MASKVAL)
        eps_sb = const.tile([128, 1], F32, name="eps")
        nc.vector.memset(eps_sb, BN_EPS)
        pk4 = const.tile([128, 2 * CT], F32, name="pk4")

        # Dummy AllReduce: forces a coordinated collective launch across the
        # 8 cores (without it the runtime serializes core executions and the
        # stats-RDMA arrival wait spans other cores' entire runs) and acts as
        # the start rendezvous for the raw RDMA protocol.  Runs concurrently
        # with compute; result unused.
        wrm_i = dpool.tile([128, 1], F32, name="wrm_i")
        wrm_o = dpool.tile([128, 1], F32, name="wrm_o")
        wrm_s = const.tile([128, 1], F32, name="wrm_s")
        nc.vector.memset(wrm_s, 0.0)
        nc.sync.dma_start(out=wrm_i, in_=wrm_s)
        for _ in range(2):
            nc.gpsimd.collective_compute(
                "AllReduce", ALU.add,
                replica_groups=[list(range(N_CORES))],
                ins=[wrm_i.opt()], outs=[wrm_o.opt()])

        stats_sb = [stats_p.tile([128, 2 * B, 6], F32, name=f"stats{m}")
                    for m in range(CT)]
        final_sb = [[None] * CT for _ in range(B)]

        # ---- per-batch compute ----
        xc_all = [None] * B

        def load_x(b):
            xc = []
            for kc in range(CT):
                xt = xin.tile([128, NPIX], FP16, name="xc")
                nc.scalar.dma_start(
                    out=xt, in_=x[b, 128 * kc:128 * (kc + 1), :])
                xc.append(xt)
            xc_all[b] = xc

        load_x(0)
        pend_kv = {}

        def emit_kv_tile(xc, t):
            """KV-projection matmuls for one n-tile + exp(k) + v cast."""
            kvp = pbig.tile([128, 1024], F32, name="kvp", tag="big")
            for kc in range(CT):
                nc.tensor.matmul(
                    kvp[:, 0:512],
                    lhsT=xc[kc][:, 128 * t:128 * (t + 1)],
                    rhs=wqkv_sb[kc][:, HID:2 * HID],
                    start=(kc == 0), stop=(kc == CT - 1))
            for kc in range(CT):
                nc.tensor.matmul(
                    kvp[:, 512:1024],
                    lhsT=xc[kc][:, 128 * t:128 * (t + 1)],
                    rhs=wqkv_sb[kc][:, 2 * HID:3 * HID],
                    start=(kc == 0), stop=(kc == CT - 1))
            ek = kpool.tile([128, HID], FP16, name="expk")
            nc.scalar.activation(out=ek, in_=kvp[:, 0:512], func=AF.Exp)
            vt = vpool.tile([128, H, D + 1], FP16, name="vx")
            nc.gpsimd.memset(vt[:, :, D:D + 1], 1.0)
            vdst = vt[:, :, 0:D]
            vsrc = kvp[:, 512:1024].rearrange("p (h e) -> p h e", h=H)
            if t % 2 == 0:
                nc.scalar.copy(vdst, vsrc)
            else:
                nc.vector.tensor_copy(vdst, vsrc)
            return ek, vt

        for b in range(B):
            xc = xc_all[b]
            pend = pend_kv.pop(b, None)
            expk = [None] * NT
            vx = [None] * NT
            ctxu = [None] * QT   # [128, 130] psum per head pair

            def ctx_mm(t, pr):
                if ctxu[pr] is None:
                    ctxu[pr] = pctx.tile([128, 2 * (D + 1)], F32,
                                         name="ctxu", tag="ctx")
                nc.tensor.matmul(
                    ctxu[pr],
                    lhsT=expk[t][:, 128 * pr:128 * (pr + 1)],
                    rhs=vx[t][:, 2 * pr:2 * (pr + 1), :],
                    start=(t == 0), stop=(t == NT - 1))

            # ---- KV projection; ctx pass A (head pairs 0,1) at lag 1 ----
            for t in range(NT):
                if pend is not None and t < len(pend):
                    expk[t], vx[t] = pend[t]
                else:
                    expk[t], vx[t] = emit_kv_tile(xc, t)
                if t >= 1:
                    ctx_mm(t - 1, 0)
                    ctx_mm(t - 1, 1)
            ctx_mm(NT - 1, 0)
            ctx_mm(NT - 1, 1)

            # prefetch next batch's x while this batch computes
            if b + 1 < B:
                load_x(b + 1)

            # ---- Q projection (1024-wide psum tiles + exp) ----
            expq = [None] * QT
            for tq in range(QT):
                qp = pbig.tile([128, 1024], F32, name="qp", tag="big")
                for nch in range(2):
                    for kc in range(CT):
                        nc.tensor.matmul(
                            qp[:, 512 * nch:512 * (nch + 1)],
                            lhsT=wqkv_sb[kc][:, 128 * tq:128 * (tq + 1)],
                            rhs=xc[kc][:, 512 * nch:512 * (nch + 1)],
                            start=(kc == 0), stop=(kc == CT - 1))
                eq = qpool.tile([128, 1024], FP16, name="expq")
                nc.scalar.activation(out=eq, in_=qp, func=AF.Exp)
                expq[tq] = eq

            # ---- ctx normalize + block-diag lhsT build (pass A pairs) ----
            bd = [None] * QT

            def extract(pr, alt):
                bdt = bdpool.tile([128, 128], FP16, name="bd")
                nc.gpsimd.memset(bdt, 0.0)
                cu = ctxu[pr]
                # the ones column multiplies every lhsT row, so ctxu[:, D]
                # already holds Zk for BOTH heads' partitions -- one
                # full-partition reciprocal (partition-offset DVE ops
                # misbehave on hardware)
                rz = small.tile([128, 1], F32, name="rz")
                nc.vector.reciprocal_approx_fast(
                    out=rz, in_=cu[:, D:D + 1])
                nc.vector.tensor_scalar_mul(
                    bdt[0:64, 0:64], in0=cu[0:64, 0:D], scalar1=rz[0:64, :])
                nc.vector.tensor_scalar_mul(
                    bdt[64:128, 64:128], in0=cu[64:128, D + 1:2 * D + 1],
                    scalar1=rz[64:128, :])
                bd[pr] = bdt

            extract(0, False)
            extract(1, True)

            # ---- ctx pass B (head pairs 2,3) ----
            ctxu[0] = ctxu[1] = None  # pass B allocates fresh ring slots
            for t in range(NT):
                ctx_mm(t, 2)
                ctx_mm(t, 3)
            extract(2, False)
            extract(3, True)

            # ---- Zq broadcast + OUT (block-diag packed) + divide ----
            out_sb = []
            for tq in range(QT):
                zqb = pbig.tile([128, 1024], F32, name="zqb", tag="big")
                outp = pbig.tile([128, 1024], F32, name="outp", tag="big")
                for nch in range(2):
                    nc.tensor.matmul(
                        zqb[:, 512 * nch:512 * (nch + 1)],
                        lhsT=bmask,
                        rhs=expq[tq][:, 512 * nch:512 * (nch + 1)],
                        start=True, stop=True)
                for nch in range(2):
                    nc.tensor.matmul(
                        outp[:, 512 * nch:512 * (nch + 1)],
                        lhsT=bd[tq],
                        rhs=expq[tq][:, 512 * nch:512 * (nch + 1)],
                        start=True, stop=True)
                # No divide in the DVE ISA, and only one operand may come
                # from PSUM: reciprocal(zqb) -> SBUF f32, then outp * recip.
                rzb = zspool.tile([128, 1024], F32, name="rzb")
                nc.vector.reciprocal_approx_fast(out=rzb, in_=zqb)
                os_ = ospool.tile([128, 1024], FP16, name="outsb")
                nc.vector.tensor_mul(os_, outp, rzb)
                out_sb.append(os_)

            # fill the FINAL-phase recip/mul stall with the next batch's
            # first two KV tiles (software pipelining across batches)
            if b + 1 < B:
                pend_kv[b + 1] = [emit_kv_tile(xc_all[b + 1], t)
                                  for t in range(2)]

            # warm the sqrt activation table once all exps are done: the BN
            # tail's Sqrt must not pay the ~1.3us act-table load
            if b == B - 1:
                warm_sq = small.tile([1, 1], F32, name="warmsq")
                nc.scalar.activation(out=warm_sq, in_=eps_sb[0:1, :],
                                     func=AF.Sqrt)

            # ---- final projection (k4-major), bn stats, psum->sbuf copy ----
            fh = [pbig.tile([128, 1024], F32, name="fh", tag="big")
                  for _ in range(CT)]
            for k4 in range(HID // 128):
                for m in range(CT):
                    for nch in range(2):
                        nc.tensor.matmul(
                            fh[m][:, 512 * nch:512 * (nch + 1)],
                            lhsT=wout_sb[k4][:, 128 * m:128 * (m + 1)],
                            rhs=out_sb[k4][:, 512 * nch:512 * (nch + 1)],
                            start=(k4 == 0), stop=(k4 == HID // 128 - 1))
            for m in range(CT):
                for nch in range(2):
                    nc.vector.bn_stats(
                        out=stats_sb[m][:, 2 * b + nch, :],
                        in_=fh[m][:, 512 * nch:512 * (nch + 1)])
                fs = fpool.tile([128, NPIX], F32, name="final")
                nc.scalar.copy(fs, fh[m])
                final_sb[b][m] = fs

        # ---- batch-norm stats: local aggregate + RDMA all-exchange ----
        for m in range(CT):
            mv = small.tile([128, 2], F32, name="mv")
            nc.vector.bn_aggr(out=mv, in_=stats_sb[m])
            pk = pk4[:, 2 * m:2 * (m + 1)]
            nc.vector.tensor_mul(pk[:, 1:2], mv[:, 0:1], mv[:, 0:1])
            nc.vector.tensor_add(pk[:, 1:2], pk[:, 1:2], mv[:, 1:2])
            nc.vector.tensor_copy(pk[:, 0:1], mv[:, 0:1])
            nc.vector.tensor_scalar_mul(pk, in0=pk, scalar1=1.0 / N_CORES)

        ccin = dpool.tile([128, 2 * CT], F32, name="ccin")
        ccout = dpool.tile([128, 2 * CT], F32, name="ccout")
        nc.sync.dma_start(out=ccin, in_=pk4)
        nc.gpsimd.collective_compute(
            "AllReduce", ALU.add,
            replica_groups=[list(range(N_CORES))],
            ins=[ccin.opt()], outs=[ccout.opt()])
        gst = small.tile([128, 2 * CT], F32, name="gst")
        nc.sync.dma_start(out=gst, in_=ccout)

        # ---- normalize + store (in-tile; Tile handles all ordering) ----
        # compute both channel-groups' affine params first, then launch all
        # eight normalizes in parallel across DVE/ACT/Pool
        rsg_t, sh_t = [], []
        for m in range(CT):
            gmean = gst[:, 2 * m:2 * m + 1]
            gex2 = gst[:, 2 * m + 1:2 * m + 2]
            var = small.tile([128, 1], F32, name="var")
            nc.vector.tensor_mul(var, gmean, gmean)
            nc.vector.tensor_sub(var, gex2, var)
            std = small.tile([128, 1], F32, name="std")
            nc.scalar.activation(out=std, in_=var, func=AF.Sqrt, bias=eps_sb)
            rstd = small.tile([128, 1], F32, name="rstd")
            nc.vector.reciprocal_approx_fast(out=rstd, in_=std)
            rsg = small.tile([128, 1], F32, name="rsg")
            nc.vector.tensor_mul(rsg, rstd, gamma_sb[m])
            sh = small.tile([128, 1], F32, name="sh")
            nc.vector.tensor_mul(sh, gmean, rsg)
            nc.vector.tensor_sub(sh, beta_sb[m], sh)
            rsg_t.append(rsg)
            sh_t.append(sh)
        k = 0
        for b in range(B):
            for m in range(CT):
                fs = final_sb[b][m]
                yv = ypool.tile([128, NPIX], FP16, name="yv")
                if k % 3 == 0:
                    nc.vector.tensor_scalar(
                        out=yv, in0=fs, scalar1=rsg_t[m], scalar2=sh_t[m],
                        op0=ALU.mult, op1=ALU.add)
                elif k % 3 == 1:
                    nc.scalar.activation(
                        out=yv, in_=fs, func=AF.Identity,
                        bias=sh_t[m], scale=rsg_t[m])
                else:
                    nc.gpsimd.tensor_scalar(
                        out=yv, in0=fs, scalar1=rsg_t[m], scalar2=sh_t[m],
                        op0=ALU.mult, op1=ALU.add)
                eng = nc.sync if k % 2 == 0 else nc.scalar
                eng.dma_start(out=y[b, 128 * m:128 * (m + 1), :], in_=yv)
                k += 1


_CACHE = {}


def _build():
    if "nc" in _CACHE:
        return _CACHE["nc"]
    nc = bacc.Bacc("TRN2", target_bir_lowering=False, debug=False,
                   enable_asserts=True, num_devices=N_CORES)
    x = nc.dram_tensor("x", [B, C, NPIX], FP16, kind="ExternalInput").ap()
    wqkv = nc.dram_tensor("wqkvT", [C, 3 * HID], FP16,
                          kind="ExternalInput").ap()
    wout = nc.dram_tensor("woutT", [HID, C], FP16, kind="ExternalInput").ap()
    gammab = nc.dram_tensor("gammab", [C, 1], F32, kind="ExternalInput").ap()
    betab = nc.dram_tensor("betab", [C, 1], F32, kind="ExternalInput").ap()
    y = nc.dram_tensor("y", [B, C, NPIX], FP16, kind="ExternalOutput").ap()

    with tile.TileContext(nc) as tc:
        _emit(tc, x, wqkv, wout, gammab, betab, y)
    nc.compile()
    _CACHE["nc"] = nc
    return nc


def kernel(x, w_qkv, w_out, b_out, gamma, beta, _trace=False):
    x = np.asarray(x, dtype=np.float32)
    wqkvT = np.ascontiguousarray(np.asarray(w_qkv, np.float16).T)   # [256, 1536]
    woutT = np.ascontiguousarray(np.asarray(w_out, np.float16).T)   # [512, 256]
    gammab = np.ascontiguousarray(np.asarray(gamma, np.float32).reshape(C, 1))
    betab = np.ascontiguousarray(np.asarray(beta, np.float32).reshape(C, 1))
    # b_out is intentionally unused: BatchNorm's mean subtraction cancels any
    # per-channel constant added before it, exactly.

    btot, c, hh, ww = x.shape
    assert (btot, c, hh * ww) == (B * N_CORES, C, NPIX)
    xf = x.reshape(btot, C, NPIX)

    nc = _build()
    in_maps = []
    for core in range(N_CORES):
        in_maps.append({
            "x": np.ascontiguousarray(xf[B * core:B * (core + 1)]).astype(np.float16),
            "wqkvT": wqkvT,
            "woutT": woutT,
            "gammab": gammab,
            "betab": betab,
        })
    res = run_bass_kernel_spmd(nc, in_maps, core_ids=list(range(N_CORES)),
                               trace=_trace)
    y = np.concatenate([res.results[core]["y"] for core in range(N_CORES)],
                       axis=0)
    out = y.reshape(btot, C, hh, ww).astype(np.float32)
    if _trace:
        kernel.last_result = res
    return out
